# revision 1
# baseline (speedup 1.0000x reference)
"""Trainium2 Bass kernel: pre-norm transformer encoder block (B=2,N=2048,D=1024,
Hid=4096,H=16 heads, raw-reshape attention merge, shared LN params).

Sharding (8 cores, no collectives):
  core c: b = c//4, heads h = 4*(c%4)..4*(c%4)+3 of batch b.
  The raw o.reshape(B,N,D) merge maps head h exactly onto tokens
  [128h, 128h+128) of the residual stream, so each core's attention output
  lands on its own contiguous 512-token slice -> MLP is token-parallel with
  zero communication.

Layout: everything transposed (features on partitions, tokens free).
  x^T is prepared on host. LayerNorm is folded into the matmuls:
      LN(x) @ W = diag(rstd) [W_eff^T x - mu * u] + bias_eff,  u = colsum(W_eff)
  so there is no normalize pass and no on-device transpose anywhere.
"""

from contextlib import ExitStack

import numpy as np
import ml_dtypes
import bass_rust
import concourse.bass as bass
import concourse.mybir as mybir
import concourse.tile as tile
from concourse.tile import TileContext, ScopedClock
from concourse.bass import ts

F32 = mybir.dt.float32
F32R = mybir.dt.float32r
BF16 = mybir.dt.bfloat16
AF = mybir.ActivationFunctionType
OP = mybir.AluOpType

B, N, D, HID, H = 2, 2048, 1024, 4096, 16
DH = D // H            # 64
NCORES = 8
CPB = 4                # cores per batch
NH = 4                 # heads per core
TOK = N                # tokens per batch (attention span)
MY = 512               # tokens owned per core (MLP/residual)
P = 128
SL = 512               # free-dim slice for matmuls
NSL = TOK // SL        # 4
KD = D // P            # 8
NKT = TOK // P         # 16
HT = HID // P          # 32
EPS = 1e-5
EXP_SHIFT = -20.0      # constant logit shift; cancels in softmax, guards overflow

_PATCHED = False


def _patch_drain():
    """This walrus build rejects >2 sem waits on one instruction; split the
    Tile kernel-tail drain's waits across single-wait NOPs."""
    global _PATCHED
    if _PATCHED:
        return
    _PATCHED = True

    def _drain_and_barrier(self, tick_clock, wait_clock):
        gc = tick_clock.global_clock
        ticks = eval(repr(gc).replace("VectorClock", ""))
        n = len(ticks)
        for i, t in enumerate(ticks):
            if t > 0:
                single = [0] * n
                single[i] = t
                vc = bass_rust.VectorClock(single)
                nop = self.nc.sync.nop(nofuse=True, hint=f"drain_split_{i}")
                wait_clock.add_sem_waits(nop.ins, ScopedClock({None: vc}))
        self.nc.sync.drain()
        self.nc.all_engine_barrier()
        assert self.sems is not None
        popped = self.nc._tile_sem_poison_stack.pop()
        assert popped is self._sem_poison
        self.nc.clear_and_free_semaphores(list(self.sems.allocated().values()))
        self.nc.all_engine_barrier()

    TileContext._drain_and_barrier = _drain_and_barrier


def _split_excess_waits(nc):
    """This walrus build supports only one sync wait per instruction (two for
    EventSemaphore). Tile emits more; move the excess onto injected NoOps that
    run just before the instruction on the same engine."""
    nid = [0]
    for fn in nc.m.functions:
        for bb in fn.blocks:
            out = []
            changed = False
            for inst in bb.instructions:
                si = inst.sync_info
                waits = list(si.on_wait) if si is not None and si.on_wait else []
                cap = 2 if inst.opcode == "EventSemaphore" else 1
                if len(waits) > cap:
                    changed = True
                    for w in waits[:-cap]:
                        nid[0] += 1
                        nop = bass_rust.InstNoOp(
                            name=f"I-wsplit{nid[0]}", ins=[], outs=[])
                        nop.engine = inst.engine
                        nop.sync_info = bass_rust.SyncInfo(
                            on_wait=[w], on_update=[])
                        out.append(nop)
                    ups = list(si.on_update) if si.on_update else []
                    inst.sync_info = bass_rust.SyncInfo(
                        on_wait=waits[-cap:], on_update=ups)
                out.append(inst)
            if changed:
                bb.instructions = out


def build_program(split_waits=True):
    _patch_drain()
    rr = lambda ap: ap.bitcast(F32R)
    nc = bass.Bass()

    xT = nc.dram_tensor("xT", [D, TOK], F32R, kind="ExternalInput")
    xTmy = nc.dram_tensor("xTmy", [D, MY], F32, kind="ExternalInput")
    wqk = nc.dram_tensor("wqk", [D, 4 * P], F32R, kind="ExternalInput")
    wv = nc.dram_tensor("wv", [D, NH * DH], F32R, kind="ExternalInput")
    uqk = nc.dram_tensor("uqk", [4 * P], F32, kind="ExternalInput")
    bqk = nc.dram_tensor("bqk", [4 * P], F32, kind="ExternalInput")
    uv = nc.dram_tensor("uv", [NH * DH], F32, kind="ExternalInput")
    bv = nc.dram_tensor("bv", [NH * DH], F32, kind="ExternalInput")
    w1 = nc.dram_tensor("w1", [D, HID], BF16, kind="ExternalInput")
    b1 = nc.dram_tensor("b1", [HID], F32, kind="ExternalInput")
    w2 = nc.dram_tensor("w2", [HID, D], BF16, kind="ExternalInput")
    b2 = nc.dram_tensor("b2", [D], F32, kind="ExternalInput")
    ones_in = nc.dram_tensor("ones_in", [P], F32R, kind="ExternalInput")
    outT = nc.dram_tensor("outT", [D, MY], F32, kind="ExternalOutput")

    # scratch: LN1 stats from a [1,TOK] free-axis row to [P,NKT] partition-major
    scr_rstd = nc.dram_tensor("scr_rstd", [TOK], F32)
    scr_musr = nc.dram_tensor("scr_musr", [TOK], F32)
    scr_rcp = nc.dram_tensor("scr_rcp", [SL], F32)
    scr2_rstd = nc.dram_tensor("scr2_rstd", [MY], F32)
    scr2_musr = nc.dram_tensor("scr2_musr", [MY], F32)

    with TileContext(nc) as tc, ExitStack() as top:
        singles = top.enter_context(tc.tile_pool(name="singles", bufs=1))
        x2T_pool = top.enter_context(tc.tile_pool(name="x2T", bufs=1))

        ones = singles.tile([P, 1], F32R)
        nc.sync.dma_start(out=ones, in_=ones_in[:, None])
        eps1 = singles.tile([1, 1], F32)
        nc.vector.memset(eps1, EPS)
        shiftP = singles.tile([P, 1], F32)
        nc.vector.memset(shiftP, EXP_SHIFT)
        b1_sb = singles.tile([P, HT], F32)
        nc.sync.dma_start(out=b1_sb, in_=b1.rearrange("(c p) -> p c", p=P))
        b2_sb = singles.tile([P, KD], F32)
        nc.sync.dma_start(out=b2_sb, in_=b2.rearrange("(c p) -> p c", p=P))

        # ================= Phase A: LN1 stats + qkv + v (sl-streamed) ========
        esAB = ExitStack()   # pools that live through phase B (qkvT, V')
        qkvT_pool = esAB.enter_context(tc.tile_pool(name="qkvT", bufs=1))
        vsb_pool = esAB.enter_context(tc.tile_pool(name="vsb", bufs=1))

        esA = ExitStack()    # phase-A only
        xTS_pool = esA.enter_context(tc.tile_pool(name="xTS", bufs=3))
        wv_pool = esA.enter_context(tc.tile_pool(name="wvp", bufs=1))
        lnA = esA.enter_context(tc.tile_pool(name="lnA", bufs=1))
        wqkS_pool = esA.enter_context(tc.tile_pool(name="wqkS", bufs=24))
        bc_pool = esA.enter_context(tc.tile_pool(name="bcA", bufs=2))

        uqk_sb = lnA.tile([P, 4], F32)
        nc.sync.dma_start(out=uqk_sb, in_=uqk.rearrange("(c p) -> p c", p=P))
        bqk_sb = lnA.tile([P, 4], F32)
        nc.sync.dma_start(out=bqk_sb, in_=bqk.rearrange("(c p) -> p c", p=P))
        uvB = lnA.tile([P, NH * DH], F32)
        nc.sync.dma_start(out=uvB, in_=uv[None, :].to_broadcast([P, NH * DH]))
        bvB = lnA.tile([P, NH * DH], F32)
        nc.sync.dma_start(out=bvB, in_=bv[None, :].to_broadcast([P, NH * DH]))

        wv_sb = []
        for k in range(KD):
            t = wv_pool.tile([P, NH * DH], F32R, tag=f"wv{k}")
            nc.sync.dma_start(out=t, in_=wv[ts(k, P), :])
            wv_sb.append(t)

        rstd_row = lnA.tile([1, TOK], F32)
        musr_row = lnA.tile([1, TOK], F32)
        rstd_col = lnA.tile([P, NKT], F32)
        musr_col = lnA.tile([P, NKT], F32)

        # qkvT col-tiles: 0=[q_h0;q_h1] 1=[q_h2;q_h3] 2=[k_h0;k_h1] 3=[k_h2;k_h3]
        qkvT = [qkvT_pool.tile([P, TOK], F32R, name=f"qkvT{ct}", tag=f"qkvT{ct}")
                for ct in range(4)]
        vsb = [[None] * NH for _ in range(NKT)]

        with (
            tc.tile_pool(name="psA", bufs=1, space="PSUM") as psA,
            tc.tile_pool(name="psQ", bufs=2, space="PSUM") as psQ,
            tc.tile_pool(name="psV", bufs=2, space="PSUM") as psV,
            tc.tile_pool(name="sq", bufs=2) as sq_pool,
            tc.tile_pool(name="rowtmp", bufs=1) as row_pool,
            tc.tile_pool(name="qkvtmp", bufs=2) as qkvtmp_pool,
        ):
            for sl in range(NSL):
                xT_sl = []
                for k in range(KD):
                    t = xTS_pool.tile([P, SL], F32R, name="xts", tag=f"xts{k}")
                    nc.sync.dma_start(out=t, in_=xT[ts(k, P), ts(sl, SL)])
                    xT_sl.append(t)

                # --- LN1 stats for this token slice ---
                s1p = psA.tile([1, SL], F32, tag="s1")
                s2p = psA.tile([1, SL], F32, tag="s2")
                for k in range(KD):
                    xf = xT_sl[k].bitcast(F32)
                    xsq = sq_pool.tile([P, SL], F32R, name="xsq", tag="xsq")
                    nc.vector.tensor_mul(xsq, xf, xf)
                    nc.tensor.matmul(s1p, lhsT=ones, rhs=xT_sl[k],
                                     start=(k == 0), stop=(k == KD - 1))
                    nc.tensor.matmul(s2p, lhsT=ones, rhs=xsq,
                                     start=(k == 0), stop=(k == KD - 1))
                mus = row_pool.tile([1, SL], F32, tag="mus")
                nc.vector.tensor_scalar_mul(mus, s1p, 1.0 / D)
                m2 = row_pool.tile([1, SL], F32, tag="m2")
                nc.vector.tensor_scalar_mul(m2, s2p, 1.0 / D)
                mu2 = row_pool.tile([1, SL], F32, tag="mu2")
                nc.vector.tensor_mul(mu2, mus, mus)
                var = row_pool.tile([1, SL], F32, tag="var")
                nc.vector.tensor_sub(var, m2, mu2)
                sd = row_pool.tile([1, SL], F32, tag="sd")
                nc.scalar.activation(out=sd, in_=var, func=AF.Sqrt,
                                     bias=eps1, scale=1.0)
                nc.vector.reciprocal(rstd_row[:, ts(sl, SL)], sd)
                mr = row_pool.tile([1, SL], F32, tag="m2")
                nc.vector.tensor_mul(mr, mus, rstd_row[:, ts(sl, SL)])
                nc.vector.tensor_scalar_mul(musr_row[:, ts(sl, SL)], mr, -1.0)

                # partition-major stats for this slice (DRAM roundtrip)
                nc.sync.dma_start(out=scr_rstd[ts(sl, SL)],
                                  in_=rstd_row[0:1, ts(sl, SL)])
                nc.sync.dma_start(out=scr_musr[ts(sl, SL)],
                                  in_=musr_row[0:1, ts(sl, SL)])
                nc.sync.dma_start(
                    out=rstd_col[:, 4 * sl:4 * sl + 4],
                    in_=scr_rstd[ts(sl, SL)].rearrange("(c p) -> p c", p=P))
                nc.sync.dma_start(
                    out=musr_col[:, 4 * sl:4 * sl + 4],
                    in_=scr_musr[ts(sl, SL)].rearrange("(c p) -> p c", p=P))

                rstdB = bc_pool.tile([P, SL], F32, tag="rstdB")
                nc.sync.dma_start(
                    out=rstdB,
                    in_=scr_rstd[ts(sl, SL)][None, :].to_broadcast([P, SL]))
                musrB = bc_pool.tile([P, SL], F32, tag="musrB")
                nc.sync.dma_start(
                    out=musrB,
                    in_=scr_musr[ts(sl, SL)][None, :].to_broadcast([P, SL]))

                # --- qkv for this slice ---
                for ct in range(4):
                    wq_sb = []
                    for k in range(KD):
                        t = wqkS_pool.tile([P, P], F32R, name="wqs", tag="wqs")
                        nc.sync.dma_start(out=t, in_=wqk[ts(k, P), ts(ct, P)])
                        wq_sb.append(t)
                    pq = psQ.tile([P, SL], F32, tag="pq")
                    for k in range(KD):
                        nc.tensor.matmul(
                            pq, lhsT=wq_sb[k], rhs=xT_sl[k],
                            start=(k == 0), stop=(k == KD - 1))
                    dst = qkvT[ct][:, ts(sl, SL)]
                    nc.vector.tensor_mul(dst, pq, rstdB)
                    r2 = qkvtmp_pool.tile([P, SL], F32, tag="r2")
                    nc.vector.tensor_scalar(
                        out=r2, in0=musrB,
                        scalar1=uqk_sb[:, ct:ct + 1],
                        scalar2=bqk_sb[:, ct:ct + 1], op0=OP.mult, op1=OP.add)
                    nc.vector.tensor_add(dst, dst.bitcast(F32), r2)

                # --- v for this slice's 4 nk tiles ---
                for nkl in range(SL // P):
                    nk = (SL // P) * sl + nkl
                    pv = psV.tile([P, NH * DH], F32, tag="pv")
                    for k in range(KD):
                        nc.tensor.matmul(
                            pv, lhsT=xT_sl[k][:, ts(nkl, P)], rhs=wv_sb[k],
                            start=(k == 0), stop=(k == KD - 1))
                    r256 = qkvtmp_pool.tile([P, NH * DH], F32, tag="r256")
                    nc.vector.tensor_scalar(
                        out=r256, in0=uvB, scalar1=musr_col[:, nk:nk + 1],
                        scalar2=None, op0=OP.mult)
                    c256 = qkvtmp_pool.tile([P, NH * DH], F32, tag="c256")
                    nc.vector.tensor_add(c256, r256, bvB)
                    sc = qkvtmp_pool.tile([P, NH * DH], F32, tag="vsc")
                    nc.scalar.activation(out=sc, in_=pv, func=AF.Copy,
                                         bias=0.0, scale=rstd_col[:, nk:nk + 1])
                    for hh in range(NH):
                        vt = vsb_pool.tile([P, DH + 1], BF16, name=f"v{nk}_{hh}",
                                           tag=f"v{nk}_{hh}")
                        nc.vector.tensor_add(vt[:, 0:DH], sc[:, ts(hh, DH)],
                                             c256[:, ts(hh, DH)])
                        nc.vector.memset(vt[:, DH:DH + 1], 1.0)
                        vsb[nk][hh] = vt

        esA.close()   # free xT stream, wv, wqk stream, LN1 vectors

        # ================= Phase B: attention =================
        x2T = [x2T_pool.tile([P, MY], F32R, name=f"x2T{k}", tag=f"x2T{k}")
               for k in range(KD)]
        with ExitStack() as esB:
            psS = esB.enter_context(tc.tile_pool(name="psS", bufs=2, space="PSUM"))
            psO = esB.enter_context(tc.tile_pool(name="psO", bufs=2, space="PSUM"))
            pT_pool = esB.enter_context(tc.tile_pool(name="pT", bufs=4))
            oT_pool = esB.enter_context(tc.tile_pool(name="oT", bufs=1))
            rcp_pool = esB.enter_context(tc.tile_pool(name="rcp", bufs=2))
            xTmy_pool = esB.enter_context(tc.tile_pool(name="xTmyp", bufs=1))

            xTmy_sb = []
            for k in range(KD):
                t = xTmy_pool.tile([P, MY], F32, tag=f"xTmy{k}")
                nc.sync.dma_start(out=t, in_=xTmy[ts(k, P), :])
                xTmy_sb.append(t)

            for pair in range(2):
                qq = qkvT[pair]
                kk = qkvT[2 + pair]
                oTs2 = [oT_pool.tile([P, TOK], F32, name=f"oTs{h}", tag=f"oT{h}")
                        for h in range(2)]
                for sl in range(NSL):
                    po2 = [psO.tile([DH + 1, SL], F32, name=f"po{h}",
                                    tag=f"po{h}") for h in range(2)]
                    for nk in range(NKT):
                        ps2 = psS.tile([P, 2 * SL], F32, name="ps2", tag="ps2")
                        nc.tensor.matmul(
                            ps2[:, 0:SL], lhsT=kk[0:64, ts(nk, P)],
                            rhs=qq[0:64, ts(sl, SL)],
                            start=True, stop=True, tile_position=(0, 0))
                        nc.tensor.matmul(
                            ps2[:, SL:2 * SL], lhsT=kk[64:128, ts(nk, P)],
                            rhs=qq[64:128, ts(sl, SL)],
                            start=True, stop=True, tile_position=(64, 0))
                        pt2 = pT_pool.tile([P, 2 * SL], BF16, name="pt2", tag="pt2")
                        nc.scalar.activation(out=pt2, in_=ps2, func=AF.Exp,
                                             bias=shiftP, scale=1.0)
                        nc.tensor.matmul(
                            po2[0], lhsT=vsb[nk][2 * pair], rhs=pt2[:, 0:SL],
                            start=(nk == 0), stop=(nk == NKT - 1))
                        nc.tensor.matmul(
                            po2[1], lhsT=vsb[nk][2 * pair + 1],
                            rhs=pt2[:, SL:2 * SL],
                            start=(nk == 0), stop=(nk == NKT - 1))
                    for h in range(2):
                        # copy out of PSUM promptly to release the bank, then
                        # divide rows 0..63 by the denominator row 64 and
                        # replicate into partitions 64..127 of oTs
                        pou = rcp_pool.tile([DH + 1, SL], F32, name="pou",
                                            tag=f"pou{h}")
                        nc.vector.tensor_copy(pou, po2[h])
                        rcp = rcp_pool.tile([1, SL], F32, tag="rcp")
                        nc.vector.reciprocal(rcp, pou[DH:DH + 1, :])
                        nc.sync.dma_start(out=scr_rcp[:], in_=rcp)
                        rcpB = rcp_pool.tile([DH, SL], F32, tag="rcpB")
                        nc.sync.dma_start(
                            out=rcpB,
                            in_=scr_rcp[None, :].to_broadcast([DH, SL]))
                        oTs = oTs2[h]
                        nc.vector.tensor_mul(oTs[0:64, ts(sl, SL)],
                                             pou[0:DH, :], rcpB)
                        nc.sync.dma_start(out=oTs[64:128, ts(sl, SL)],
                                          in_=oTs[0:64, ts(sl, SL)])
                # scatter both heads' outputs into x2T via strided views:
                # attn_out^T[64j+d, m] = oT[d, 16m+j]
                for h in range(2):
                    hh = 2 * pair + h
                    c0 = P * hh
                    ov = oTs2[h].rearrange("p (m j) -> p m j", j=16)
                    for k in range(KD):
                        nc.vector.tensor_add(
                            x2T[k][0:64, c0:c0 + P],
                            xTmy_sb[k][0:64, c0:c0 + P],
                            ov[0:64, :, 2 * k])
                        nc.vector.tensor_add(
                            x2T[k][64:128, c0:c0 + P],
                            xTmy_sb[k][64:128, c0:c0 + P],
                            ov[64:128, :, 2 * k + 1])
        esAB.close()  # free qkvT, V'

        # ================= Phase C: LN2 + MLP =================
        ln2 = top.enter_context(tc.tile_pool(name="ln2", bufs=1))
        x2b_pool = top.enter_context(tc.tile_pool(name="x2b", bufs=1))
        rstd2B = ln2.tile([P, MY], F32)
        musr2B = ln2.tile([P, MY], F32)
        with (
            tc.tile_pool(name="psL", bufs=1, space="PSUM") as psL,
            tc.tile_pool(name="sq2", bufs=2) as sq2_pool,
            tc.tile_pool(name="row2", bufs=1) as row2_pool,
        ):
            s1p = psL.tile([1, MY], F32, tag="s1")
            s2p = psL.tile([1, MY], F32, tag="s2")
            for k in range(KD):
                xf = x2T[k].bitcast(F32)
                xsq = sq2_pool.tile([P, MY], F32R, name="xsq2", tag="xsq2")
                nc.vector.tensor_mul(xsq, xf, xf)
                nc.tensor.matmul(s1p, lhsT=ones, rhs=x2T[k],
                                 start=(k == 0), stop=(k == KD - 1))
                nc.tensor.matmul(s2p, lhsT=ones, rhs=xsq,
                                 start=(k == 0), stop=(k == KD - 1))
            mu2r = row2_pool.tile([1, MY], F32, tag="mu2r")
            nc.vector.tensor_scalar_mul(mu2r, s1p, 1.0 / D)
            m2 = row2_pool.tile([1, MY], F32, tag="m2b")
            nc.vector.tensor_scalar_mul(m2, s2p, 1.0 / D)
            mu22 = row2_pool.tile([1, MY], F32, tag="mu22")
            nc.vector.tensor_mul(mu22, mu2r, mu2r)
            var = row2_pool.tile([1, MY], F32, tag="var2")
            nc.vector.tensor_sub(var, m2, mu22)
            sd = row2_pool.tile([1, MY], F32, tag="sd2")
            nc.scalar.activation(out=sd, in_=var, func=AF.Sqrt,
                                 bias=eps1, scale=1.0)
            rstd2r = row2_pool.tile([1, MY], F32, tag="rstd2r")
            nc.vector.reciprocal(rstd2r, sd)
            mr2 = row2_pool.tile([1, MY], F32, tag="m2b")
            nc.vector.tensor_mul(mr2, mu2r, rstd2r)
            musr2r = row2_pool.tile([1, MY], F32, tag="musr2r")
            nc.vector.tensor_scalar_mul(musr2r, mr2, -1.0)
            nc.sync.dma_start(out=scr2_rstd[:], in_=rstd2r)
            nc.sync.dma_start(out=scr2_musr[:], in_=musr2r)
            nc.sync.dma_start(out=rstd2B,
                              in_=scr2_rstd[None, :].to_broadcast([P, MY]))
            nc.sync.dma_start(out=musr2B,
                              in_=scr2_musr[None, :].to_broadcast([P, MY]))

        # materialize normalized xn2 = x2*rstd - mu*rstd in bf16 for the MLP
        # (ln_g/ln_b are folded into w1/b1 on the host)
        x2b = []
        for k in range(KD):
            xh = x2b_pool.tile([P, MY], F32, name=f"xh{k}", tag="xh")
            nc.vector.tensor_mul(xh, x2T[k].bitcast(F32), rstd2B)
            t = x2b_pool.tile([P, MY], BF16, name=f"x2b{k}", tag=f"x2b{k}")
            nc.vector.tensor_add(t, xh, musr2B)
            x2b.append(t)

        with (
            tc.tile_pool(name="psF", bufs=2, space="PSUM") as psF,
            tc.tile_pool(name="w1sb", bufs=1) as w1_pool,
            tc.tile_pool(name="hT", bufs=1) as hT_pool,
            tc.tile_pool(name="fctmp", bufs=2) as fctmp_pool,
        ):
            hT = [None] * HT
            GK = 8           # hid col groups of 512
            GW = HID // GK   # 512
            for gk in range(GK):
                w1sb = []
                for k in range(KD):
                    t = w1_pool.tile([P, GW], BF16, name="w1t",
                                     tag=f"w1_{k}_{gk % 2}")
                    nc.sync.dma_start(out=t, in_=w1[ts(k, P), ts(gk, GW)])
                    w1sb.append(t)
                for khl in range(GW // P):
                    kh = (GW // P) * gk + khl
                    pf = psF.tile([P, MY], F32, tag="pf")
                    for k in range(KD):
                        nc.tensor.matmul(
                            pf, lhsT=w1sb[k][:, ts(khl, P)], rhs=x2b[k],
                            start=(k == 0), stop=(k == KD - 1))
                    ht = hT_pool.tile([P, MY], BF16, name="ht", tag=f"hT{kh}")
                    nc.scalar.activation(out=ht, in_=pf, func=AF.Gelu,
                                         bias=b1_sb[:, kh:kh + 1], scale=1.0)
                    hT[kh] = ht

            with tc.tile_pool(name="w2sb", bufs=2) as w2_pool:
                w2r = w2.rearrange("(c p) d -> p c d", p=P)   # [128, 32, 1024]
                for kd in range(KD):
                    pf = psF.tile([P, MY], F32, tag="pf2")
                    for half in range(2):
                        w2h = w2_pool.tile([P, HT // 2, P], BF16, name="w2t",
                                           tag="w2sb")
                        nc.sync.dma_start(
                            out=w2h,
                            in_=w2r[:, ts(half, HT // 2), ts(kd, P)])
                        for khl in range(HT // 2):
                            kh = half * (HT // 2) + khl
                            nc.tensor.matmul(
                                pf, lhsT=w2h[:, khl, :], rhs=hT[kh],
                                start=(kh == 0), stop=(kh == HT - 1))
                    t = fctmp_pool.tile([P, MY], F32, tag="fco")
                    nc.scalar.activation(out=t, in_=pf, func=AF.Identity,
                                         bias=b2_sb[:, kd:kd + 1], scale=1.0)
                    ot = fctmp_pool.tile([P, MY], F32, tag="fcout")
                    nc.vector.tensor_add(ot, t, x2T[kd].bitcast(F32))
                    nc.sync.dma_start(out=outT[ts(kd, P), :], in_=ot)

    if split_waits:
        _split_excess_waits(nc)
    return nc


def host_prep(x, w_qkv, b_qkv, ln_g, ln_b, w1, b1, w2, b2):
    """Fold LN affine params into weights; build per-core input maps."""
    x = np.asarray(x, np.float32)
    w_qkv = np.asarray(w_qkv, np.float32)
    b_qkv = np.asarray(b_qkv, np.float32)
    ln_g = np.asarray(ln_g, np.float32)
    ln_b = np.asarray(ln_b, np.float32)
    w1 = np.asarray(w1, np.float32)
    b1 = np.asarray(b1, np.float32)
    w2 = np.asarray(w2, np.float32)
    b2 = np.asarray(b2, np.float32)

    wqkv_eff = ln_g[:, None] * w_qkv
    bqkv_eff = b_qkv + ln_b @ w_qkv
    w1_eff = np.ascontiguousarray(ln_g[:, None] * w1)
    b1_eff = b1 + ln_b @ w1
    u_qkv = wqkv_eff.sum(axis=0)

    in_maps = []
    for c in range(NCORES):
        b = c // CPB
        heads = [4 * (c % CPB) + i for i in range(NH)]
        qcols = np.concatenate([np.arange(h * DH, (h + 1) * DH) for h in heads])
        kcols = qcols + D
        vcols = qcols + 2 * D
        qkcols = np.concatenate([qcols, kcols])
        xb = x[b]
        my0 = MY * (c % CPB)
        in_maps.append({
            "ones_in": np.ones(P, np.float32),
            "xT": np.ascontiguousarray(xb.T),
            "xTmy": np.ascontiguousarray(xb[my0:my0 + MY].T),
            "wqk": np.ascontiguousarray(wqkv_eff[:, qkcols]),
            "wv": np.ascontiguousarray(wqkv_eff[:, vcols]),
            "uqk": np.ascontiguousarray(u_qkv[qkcols]),
            "bqk": np.ascontiguousarray(bqkv_eff[qkcols]),
            "uv": np.ascontiguousarray(u_qkv[vcols]),
            "bv": np.ascontiguousarray(bqkv_eff[vcols]),
            "w1": w1_eff.astype(ml_dtypes.bfloat16),
            "b1": b1_eff,
            "w2": w2.astype(ml_dtypes.bfloat16),
            "b2": b2,
        })
    return in_maps


_NC_CACHE = None


def kernel(x, w_qkv, b_qkv, ln_g, ln_b, w1, b1, w2, b2):
    global _NC_CACHE
    from concourse.bass_utils import run_bass_kernel_spmd

    if _NC_CACHE is None:
        _NC_CACHE = build_program()
    nc = _NC_CACHE
    in_maps = host_prep(x, w_qkv, b_qkv, ln_g, ln_b, w1, b1, w2, b2)
    res = run_bass_kernel_spmd(nc, in_maps, list(range(NCORES))).results

    out = np.empty((B, N, D), np.float32)
    for c in range(NCORES):
        b = c // CPB
        my0 = MY * (c % CPB)
        out[b, my0:my0 + MY, :] = res[c]["outT"].T
    return out

